# revision 3
# baseline (speedup 1.0000x reference)
"""Trainium2 Bass kernel for nn_CharRNN: bidirectional char-GRU + temporal max-pool.

Problem shapes (hardcoded): B=64, S=256, T=16, V=262, E=64, H=32.
16384 independent char sequences ("words") are sharded 8 ways (2048 words/core).

Design: two staggered per-direction chains (the latency-optimal structure), with:
- SEPARATE h_f / h_b state tiles so each dir's next-step matmuls wait only on
  its own h' (a shared pair tile couples the chains at tile granularity).
- Packed constants (one wpack DMA + one bias DMA) and packed gi tiles (one
  giRZ + one giN DMA per step) so the prologue is 2 issues deep, not 9, and
  the identity matrix lands before the first injection.
- gi tiles prefetched PREFETCH steps ahead on the fat gpsimd SWDGE queue.
- gi_r|gi_z PE-injected into PSUM (identity matmuls, off the critical path);
  hh matmuls accumulate (start=False).  Step 0 skips all hh matmuls (h0 == 0).

Per-step math (PyTorch GRU cell, biases pre-folded into gi):
  r = sigmoid(gi_r + Whh_r h)          gi_r includes b_ih_r + b_hh_r
  z = sigmoid(gi_z + Whh_z h)          gi_z includes b_ih_z + b_hh_z
  n = tanh(gi_n + r*(Whh_n h + b_hh_n))   gi_n includes b_ih_n
  h' = n + z*(h - n);  ymax = max(ymax, h')
"""

import sys
import os

sys.path.insert(0, "/opt/trn_rl_repo")

import numpy as np

import concourse.bacc as bacc
import concourse.tile as tile
from concourse import mybir
from concourse.bass_utils import run_bass_kernel_spmd
from concourse.alu_op_type import AluOpType as Alu

B, S, T = 64, 256, 16
V, E, H = 262, 64, 32
NCORES = 8
WPC = 16384 // NCORES  # words per core = 2048
NG = 4                 # word groups per direction chain
GW = WPC // NG         # words per group = free width = 512

F32 = mybir.dt.float32
BF16 = mybir.dt.bfloat16
AF = mybir.ActivationFunctionType

DIRS = ("f", "b")
PREFETCH = 3

_CACHE = {}


def _build_program():
    nc = bacc.Bacc("TRN2", target_bir_lowering=False, debug=False, num_devices=NCORES)

    d_giRZ = nc.dram_tensor("giRZ", [T, 128, 4 * GW], BF16, kind="ExternalInput").ap()
    d_giN = nc.dram_tensor("giN", [T, 128, 2 * GW], BF16, kind="ExternalInput").ap()
    # wpack: [ident | hhR_f | hhZ_f | hhN_f | hhR_b | hhZ_b | hhN_b]
    d_wpack = nc.dram_tensor("wpack", [128, 7 * 128], BF16, kind="ExternalInput").ap()
    d_bhh = nc.dram_tensor("bhhN", [128, 2], F32, kind="ExternalInput").ap()
    d_out = nc.dram_tensor("out", [128, 2 * GW], BF16, kind="ExternalOutput").ap()

    with tile.TileContext(nc) as tc:
        with (
            tc.tile_pool(name="consts", bufs=1) as consts,
            tc.tile_pool(name="gi", bufs=PREFETCH + 1) as gip,
            tc.tile_pool(name="state", bufs=1) as state,
            tc.tile_pool(name="work", bufs=2) as work,
            tc.tile_pool(name="psRZ", bufs=1, space="PSUM") as psRZ,
            tc.tile_pool(name="psN", bufs=1, space="PSUM") as psN,
        ):
            # ---- constants: 2 DMA issues, land before the first inject ----
            wpack = consts.tile([128, 7 * 128], BF16, name="wpack")
            nc.sync.dma_start(out=wpack, in_=d_wpack)
            s_bhhpk = consts.tile([128, 2], F32, name="bhh")
            nc.sync.dma_start(out=s_bhhpk, in_=d_bhh)
            s_id = wpack[:, 0:128]
            s_hh = {}
            for di, d in enumerate(DIRS):
                for gi_, g in enumerate("RZN"):
                    s_hh[(d, g)] = wpack[:, (1 + 3 * di + gi_) * 128:(2 + 3 * di + gi_) * 128]
            s_bhh = {d: s_bhhpk[:, i:i + 1] for i, d in enumerate(DIRS)}

            # ---- state ----
            hz = state.tile([128, 2 * GW], BF16, name="h0")
            nc.vector.memset(hz, 0.0)
            h = {d: hz[:, i * GW:(i + 1) * GW] for i, d in enumerate(DIRS)}
            ymax = state.tile([128, 2 * GW], BF16, name="ymax")
            ycol = {d: slice(i * GW, (i + 1) * GW) for i, d in enumerate(DIRS)}

            # ---- gi prefetch: one DMA issue per tile ----
            giRZ, giN = {}, {}

            def fetch(s, rz_eng=None):
                # early tiles go out on the sync/Q1 queue so the step-0 giRZ
                # has the fat gpsimd/Q0 queue to itself (the DMA engines
                # fair-share across queued transfers within a queue).
                giRZ[s] = gip.tile([128, 4 * GW], BF16, tag="giRZ", name=f"giRZ{s}")
                (rz_eng or nc.gpsimd).dma_start(out=giRZ[s], in_=d_giRZ[s])
                giN[s] = gip.tile([128, 2 * GW], BF16, tag="giN", name=f"giN{s}")
                nc.sync.dma_start(out=giN[s], in_=d_giN[s])

            giRZ_off = {"f": 0, "b": 2 * GW}
            giN_col = {"f": slice(0, GW), "b": slice(GW, 2 * GW)}
            R, Z = slice(0, GW), slice(GW, 2 * GW)

            pR, pZ, pN = {}, {}, {}

            def inject_rz(s):
                # separate R / Z PSUM tiles: the R-injection for step s+1 then
                # waits only on sigmoid-R of step s (tile-granularity WAR),
                # not on sigmoid-Z too.
                last = s == 0
                for d in DIRS:
                    off = giRZ_off[d]
                    pR[d] = psRZ.tile([128, GW], F32, tag=f"r{d}", name=f"pR_{d}{s}")
                    nc.tensor.matmul(pR[d], lhsT=s_id,
                                     rhs=giRZ[s][:, off:off + GW], start=True, stop=last)
                    pZ[d] = psRZ.tile([128, GW], F32, tag=f"z{d}", name=f"pZ_{d}{s}")
                    nc.tensor.matmul(pZ[d], lhsT=s_id,
                                     rhs=giRZ[s][:, off + GW:off + 2 * GW], start=True, stop=last)

            fetch(0)
            inject_rz(0)

            for s in range(T):
                # PE: hh accumulations (critical: R first), f then b.
                for d in DIRS:
                    if s > 0:
                        pN[d] = psN.tile([128, GW], F32, tag=f"n{d}", name=f"pN_{d}{s}")
                        nc.tensor.matmul(pR[d], lhsT=s_hh[(d, "R")], rhs=h[d], start=False, stop=True)
                        nc.tensor.matmul(pZ[d], lhsT=s_hh[(d, "Z")], rhs=h[d], start=False, stop=True)
                        nc.tensor.matmul(pN[d], lhsT=s_hh[(d, "N")], rhs=h[d], start=True, stop=True)

                pR_s, pZ_s = dict(pR), dict(pZ)
                # ACT: all four sigmoids FIRST so tanh_f never blocks the
                # b-chain's sigmoids in the in-order ACT queue.
                rz, npre, n = {}, {}, {}
                for d in DIRS:
                    rz[d] = work.tile([128, 2 * GW], BF16, tag=f"rzs{d}", name=f"rz_{d}{s}")
                    nc.scalar.activation(rz[d][:, R], pR_s[d], AF.Sigmoid)
                    nc.scalar.activation(rz[d][:, Z], pZ_s[d], AF.Sigmoid)
                for d in DIRS:
                    # DVE: hn = (psumN + bhh_n) * r ; npre = hn + gi_n
                    hn = work.tile([128, GW], BF16, tag=f"hn{d}", name=f"hn_{d}{s}")
                    if s > 0:
                        nc.vector.scalar_tensor_tensor(
                            out=hn, in0=pN[d], scalar=s_bhh[d], in1=rz[d][:, R],
                            op0=Alu.add, op1=Alu.mult,
                        )
                    else:  # psumN == 0: hn = bhh_n * r
                        nc.vector.tensor_scalar(
                            out=hn, in0=rz[d][:, R], scalar1=s_bhh[d],
                            scalar2=None, op0=Alu.mult,
                        )
                    npre[d] = work.tile([128, GW], BF16, tag=f"npre{d}", name=f"npre_{d}{s}")
                    nc.vector.tensor_tensor(out=npre[d], in0=hn, in1=giN[s][:, giN_col[d]], op=Alu.add)
                    # ACT: tanh
                    n[d] = work.tile([128, GW], BF16, tag=f"n{d}", name=f"n_{d}{s}")
                    nc.scalar.activation(n[d], npre[d], AF.Tanh)

                # prefetch + pre-inject for later steps (PE slots after this
                # step's sigmoids have drained the RZ banks)
                if s == 0:
                    for sp in range(1, PREFETCH + 1):
                        fetch(sp, rz_eng=nc.sync)
                elif s + PREFETCH < T:
                    fetch(s + PREFETCH)
                if s + 1 < T:
                    inject_rz(s + 1)

                # DVE tail per dir into SEPARATE h tiles (chain decoupling);
                # the off-chain ymax updates go LAST so they never delay dd_b.
                hNew = {}
                for d in DIRS:
                    dd = work.tile([128, GW], BF16, tag=f"d{d}", name=f"d_{d}{s}")
                    nc.vector.tensor_tensor(out=dd, in0=h[d], in1=n[d], op=Alu.subtract)
                    e = work.tile([128, GW], BF16, tag=f"e{d}", name=f"e_{d}{s}")
                    nc.vector.tensor_tensor(out=e, in0=rz[d][:, Z], in1=dd, op=Alu.mult)
                    hNew[d] = work.tile([128, GW], BF16, tag=f"h{d}", name=f"h_{d}{s}")
                    nc.vector.tensor_tensor(out=hNew[d], in0=n[d], in1=e, op=Alu.add)
                for d in DIRS:
                    if s == 0:
                        nc.vector.tensor_copy(out=ymax[:, ycol[d]], in_=hNew[d])
                    else:
                        nc.vector.tensor_tensor(out=ymax[:, ycol[d]], in0=ymax[:, ycol[d]], in1=hNew[d], op=Alu.max)
                    h[d] = hNew[d]

            # output DMA split across both queues to shorten the tail
            nc.sync.dma_start(out=d_out[:, 0:GW], in_=ymax[:, 0:GW])
            nc.gpsimd.dma_start(out=d_out[:, GW:2 * GW], in_=ymax[:, GW:2 * GW])

    nc.compile()
    return nc


def _prep_inputs(x, emb, Wih_f, Whh_f, bih_f, bhh_f, Wih_b, Whh_b, bih_b, bhh_b):
    """Host-side: projected-table gather of per-step gi tiles + weight packing."""
    import ml_dtypes

    f32 = np.float32
    bf16 = ml_dtypes.bfloat16
    x_flat = np.asarray(x).reshape(16384, T).astype(np.int32)

    embf = np.asarray(emb, f32)

    def proj_table(Wih, bih, bhh):
        Wih, bih, bhh = np.asarray(Wih, f32), np.asarray(bih, f32), np.asarray(bhh, f32)
        P = embf @ Wih.T  # [V, 96] (gates r,z,n)
        P[:, 0:H] += bih[0:H] + bhh[0:H]
        P[:, H:2 * H] += bih[H:2 * H] + bhh[H:2 * H]
        P[:, 2 * H:] += bih[2 * H:]
        return P.astype(bf16)

    Ptab = {"f": proj_table(Wih_f, bih_f, bhh_f), "b": proj_table(Wih_b, bih_b, bhh_b)}

    def hh_tile(Whh, gate):
        L = np.zeros((128, 128), f32)
        Wg = np.asarray(Whh, f32)[gate * H:(gate + 1) * H, :]  # [32, 32]
        for g in range(NG):
            L[g * H:(g + 1) * H, g * H:(g + 1) * H] = Wg.T
        return L

    blocks = [np.eye(128, dtype=f32)]
    for Whh in (Whh_f, Whh_b):
        for gi_ in range(3):
            blocks.append(hh_tile(Whh, gi_))
    wpack = np.concatenate(blocks, axis=1).astype(bf16)  # [128, 896]

    bhhN = np.empty((128, 2), f32)
    for i, bhh in enumerate((bhh_f, bhh_b)):
        bhhN[:, i] = np.tile(np.asarray(bhh, f32)[2 * H:], NG)

    in_maps = []
    for core in range(NCORES):
        xc = x_flat[core * WPC:(core + 1) * WPC]      # [2048, 16]
        xg = xc.reshape(NG, GW, T)                    # [4, 512, 16]
        giRZ = np.empty((T, 128, 4 * GW), bf16)
        giN = np.empty((T, 128, 2 * GW), bf16)
        for di, d in enumerate(DIRS):
            ch = xg if d == "f" else xg[:, :, ::-1]
            gathered = Ptab[d][ch]                    # [4, 512, 16, 96] bf16
            gt = np.ascontiguousarray(gathered.transpose(2, 3, 0, 1))  # [16,96,4,512]
            gt = gt.reshape(T, 3, H, NG, GW).transpose(0, 1, 3, 2, 4)  # [16,3,4,32,512]
            gt = gt.reshape(T, 3, 128, GW)
            giRZ[:, :, 2 * di * GW:(2 * di + 1) * GW] = gt[:, 0]
            giRZ[:, :, (2 * di + 1) * GW:(2 * di + 2) * GW] = gt[:, 1]
            giN[:, :, di * GW:(di + 1) * GW] = gt[:, 2]
        m = {
            "giRZ": np.ascontiguousarray(giRZ),
            "giN": np.ascontiguousarray(giN),
            "wpack": wpack,
            "bhhN": bhhN,
        }
        in_maps.append(m)
    return in_maps


def _install_ntff_hook():
    """Register the axon NTFF profiling hook (the image's antenv lacks
    axon_hooks, so run_bass_kernel_spmd's trace path can't find it)."""
    import types
    import antenv

    if "antenv.axon_hooks" in sys.modules:
        return
    mod = types.ModuleType("antenv.axon_hooks")
    _h = {"hook": None}
    mod.set_axon_ntff_profile_hook = lambda h: _h.update(hook=h)
    mod.get_axon_ntff_profile_hook = lambda: _h["hook"]
    sys.modules["antenv.axon_hooks"] = mod
    antenv.axon_hooks = mod
    try:
        from trn_agent_boot.trn_boot import _ntff_profile_via_ctypes

        hook = _ntff_profile_via_ctypes("/opt/axon/libaxon_pjrt.so")
        if hook is not None:
            mod.set_axon_ntff_profile_hook(hook)
    except Exception as e:  # profiling is best-effort
        print("ntff hook install failed:", e)
    import concourse.bass_utils as bu

    bu.upload_artifacts = lambda tmpdir: tmpdir


def kernel(x, emb, Wih_f, Whh_f, bih_f, bhh_f, Wih_b, Whh_b, bih_b, bhh_b):
    if "nc" not in _CACHE:
        _CACHE["nc"] = _build_program()
    nc = _CACHE["nc"]

    in_maps = _prep_inputs(
        x, emb, Wih_f, Whh_f, bih_f, bhh_f, Wih_b, Whh_b, bih_b, bhh_b
    )

    trace = bool(int(os.environ.get("CHAR_RNN_TRACE", "0")))
    if trace:
        _install_ntff_hook()
    res = run_bass_kernel_spmd(
        nc, in_maps, core_ids=list(range(NCORES)), trace=trace,
        trace_cores=[0] if trace else None,
    )
    _CACHE["last_results"] = res

    out = np.empty((16384, 2 * H), np.float32)
    for core in range(NCORES):
        base = core * WPC
        o_all = res.results[core]["out"].astype(np.float32)  # [128, 1024]
        for col, lo in ((0, 0), (H, GW)):
            o = o_all[:, lo:lo + GW]
            # o[32*g + dim, w] -> out[base + 512*g + w, dim]
            o = o.reshape(NG, H, GW).transpose(0, 2, 1).reshape(WPC, H)
            out[base:base + WPC, col:col + H] = o
    return out.reshape(B, S, 2 * H)
